# revision 1
# baseline (speedup 1.0000x reference)
"""Self-contained Trainium2 Bass kernel for the 3-layer GCN (AgriGraphGCN).

kernel(**inputs) -> (100000, 1) float32 risk scores, computed SPMD on 8
NeuronCores. Strategy: shard nodes by dst owner; per layer build a bf16
feature table (project + PE-transpose + degree scaling), AllGather it,
dma_gather per-edge source rows (4 SWDGE queues), scatter via one-hot PE
matmuls into PSUM accumulators, BatchNorm from AllReduce'd masked stats.
Self-loops are applied as identity matmuls over the local table shard.
"""
import sys
sys.path.insert(0, "/opt/trn_rl_repo")

"""GCN (3-layer, symmetric-norm) distributed Bass kernel for 8 TRN2 NeuronCores.

Math (per reference):
  dis = 1/sqrt(deg), deg = 1 + indeg (self-loops)
  conv(h, W) = dis * segsum_dst(m_tilde[src]) + b,  m_tilde = dis * (h @ W)
  (self-loops included as explicit edges; b1/b2 drop out under BN)
  L1: relu(bn(conv(x, W1)));  L2: relu(bn(conv(h1, W2)));  L3: sigmoid(conv(h2, W3))

Distribution: nodes sharded over 8 cores (dst-owner). Per layer:
  phase A: m_tildeT = W^T @ hT (feat-major), transpose per 128-tile via PE,
           scale by dis -> node-major bf16 table shard -> AllGather -> full table
  phase C: dma_gather per-edge rows (src) from full table; scatter via
           one-hot (DVE is_equal) matmuls accumulating into PSUM per dst-tile
  phase D: psum -> dis-scale -> BN stats (mask-matmul) -> AllReduce ->
           affine+relu in feat-major (transposed via PE)
"""
import numpy as np
import ml_dtypes


def make_cfg(full=True):
    if full:
        return dict(N=100000, E=640000, NPC=12500, NL=12544, CHUNK=32768, G=14, IN=6)
    return dict(N=4096, E=16384, NPC=512, NL=512, CHUNK=1024, G=2, IN=6)


C = 8           # cores
H = 128         # hidden
NI_MAX = 1024   # dma_gather per-call limit
BN_EPS = 1e-5


def host_prep(cfg, edge_index):
    """Build common (cross-core) batch/op structure + per-core index data."""
    N, NPC, NL, CHUNK, G = cfg["N"], cfg["NPC"], cfg["NL"], cfg["CHUNK"], cfg["G"]
    TILES = NL // 128
    NGRP = (TILES + G - 1) // G
    NCHUNK = (C * NL + CHUNK - 1) // CHUNK

    src = np.asarray(edge_index[0], dtype=np.int64)
    dst = np.asarray(edge_index[1], dtype=np.int64)
    deg = np.bincount(dst, minlength=N).astype(np.float32) + 1.0

    src_all = src
    dst_all = dst

    owner = dst_all // NPC
    dst_loc = (dst_all - owner * NPC).astype(np.int64)
    trow = (src_all // NPC) * NL + (src_all % NPC)
    chunk = trow // CHUNK
    gidx_loc = (trow - chunk * CHUNK).astype(np.int64)
    dtile = dst_loc // 128
    grp = dtile // G

    # per (core, grp, chunk): sorted edge lists
    seg_edges = {}
    for c in range(C):
        m_c = owner == c
        for g in range(NGRP):
            m_g = m_c & (grp == g)
            for k in range(NCHUNK):
                m = m_g & (chunk == k)
                idx = np.nonzero(m)[0]
                order = np.argsort(dst_loc[idx], kind="stable")
                idx = idx[order]
                seg_edges[(c, g, k)] = idx

    # common segment sizes
    seg_size = {}
    for g in range(NGRP):
        for k in range(NCHUNK):
            mx = max(len(seg_edges[(c, g, k)]) for c in range(C))
            seg_size[(g, k)] = ((mx + 127) // 128) * 128

    # batches + ops (common structure)
    batches = []   # (grp, chunk, size, seg_off) ; seg_off = start pos within segment
    ops = []       # (batch_id, col, gtile)
    for g in range(NGRP):
        for k in range(NCHUNK):
            S = seg_size[(g, k)]
            # subchunk spans (union over cores)
            nsub = S // 128
            lo = np.full(nsub, 10 ** 9, dtype=np.int64)
            hi = np.full(nsub, -1, dtype=np.int64)
            for c in range(C):
                idx = seg_edges[(c, g, k)]
                if len(idx) == 0:
                    continue
                dt = dtile[idx]
                for s in range((len(idx) + 127) // 128):
                    a, b = s * 128, min((s + 1) * 128, len(idx))
                    lo[s] = min(lo[s], dt[a:b].min())
                    hi[s] = max(hi[s], dt[a:b].max())
            off = 0
            while off < S:
                size = min(NI_MAX, S - off)
                b_id = len(batches)
                batches.append((g, k, size, off))
                for col in range(size // 128):
                    s = (off // 128) + col
                    if hi[s] < 0:
                        # no core has real edges here (can't happen for max core,
                        # but guard): emit one op on the group's first tile
                        lo[s] = hi[s] = g * G
                    for t in range(int(lo[s]), int(hi[s]) + 1):
                        ops.append((b_id, col, t))
                off += size

    NBAT, NOPS = len(batches), len(ops)
    # per-tile first/last op flags
    first_op, last_op = {}, {}
    for i, (_, _, t) in enumerate(ops):
        if t not in first_op:
            first_op[t] = i
        last_op[t] = i

    # per-core index data
    def wrap16_rep(vals, ncols):
        # idx position i -> row i%16, col i//16; replicate to 128 partitions
        a = np.zeros(16 * ncols, dtype=np.int16)
        a[: len(vals)] = vals
        w = a.reshape(ncols, 16).T
        return np.tile(w, (8, 1))  # (128, ncols)

    col_off = []
    acc_cols = 0
    for (g, k, size, off) in batches:
        col_off.append(acc_cols)
        acc_cols += size // 16
    gidx_data = np.zeros((C, 128, acc_cols), dtype=np.int16)
    dloc_data = np.full((C, 128, NOPS), -1000.0, dtype=np.float32)
    for c in range(C):
        for b_id, (g, k, size, off) in enumerate(batches):
            idx = seg_edges[(c, g, k)]
            pos = idx[off: off + size]
            gi = np.zeros(size, dtype=np.int16)
            gi[: len(pos)] = gidx_loc[pos].astype(np.int16)
            co = col_off[b_id]
            gidx_data[c, :, co: co + size // 16] = wrap16_rep(gi, size // 16)
    for c in range(C):
        for o_id, (b_id, col, t) in enumerate(ops):
            g, k, size, off = batches[b_id]
            idx = seg_edges[(c, g, k)]
            a = off + col * 128
            pos = idx[a: a + 128]
            if len(pos):
                v = dst_loc[pos].astype(np.float32) - t * 128.0
                dloc_data[c, : len(pos), o_id] = v

    return dict(
        deg=deg, batches=batches, ops=ops, first_op=first_op, last_op=last_op,
        gidx_data=gidx_data, dloc_data=dloc_data, col_off=col_off,
        GIDX_COLS=acc_cols,
        TILES=TILES, NGRP=NGRP, NCHUNK=NCHUNK, NBAT=NBAT, NOPS=NOPS,
    )


def build_graph(cfg, prep, params, num_msg_bufs=26, debug_dumps=False):
    """Build the SPMD Bass graph. params: numpy dict (W1,W2,W3,b3,g1,be1,g2,be2)."""
    import sys
    sys.path.insert(0, "/opt/trn_rl_repo")
    from concourse import bacc, tile
    import concourse.mybir as mybir

    N, NPC, NL, CHUNK, G, IN = (cfg[k] for k in ["N", "NPC", "NL", "CHUNK", "G", "IN"])
    TILES, NGRP, NCHUNK = prep["TILES"], prep["NGRP"], prep["NCHUNK"]
    NBAT, NOPS = prep["NBAT"], prep["NOPS"]
    batches, ops = prep["batches"], prep["ops"]
    col_off, GIDX_COLS = prep["col_off"], prep["GIDX_COLS"]
    first_op, last_op = prep["first_op"], prep["last_op"]
    TBLROWS = C * NL
    f32, bf16, i16 = mybir.dt.float32, mybir.dt.bfloat16, mybir.dt.int16

    nc = bacc.Bacc("TRN2", target_bir_lowering=False, num_swdge_queues=4)

    # ---- DRAM I/O ----
    xT_d = nc.dram_tensor("xT", [IN, NL], bf16, kind="ExternalInput")
    degc_d = nc.dram_tensor("degc", [128, TILES], f32, kind="ExternalInput")
    maskc_d = nc.dram_tensor("maskc", [128, TILES], bf16, kind="ExternalInput")
    gidx_d = nc.dram_tensor("gidx", [128, GIDX_COLS], i16, kind="ExternalInput")
    dloc_d = nc.dram_tensor("dloc", [128, NOPS], bf16, kind="ExternalInput")
    W1_d = nc.dram_tensor("W1", [IN, H], bf16, kind="ExternalInput")
    W2_d = nc.dram_tensor("W2", [H, H], bf16, kind="ExternalInput")
    W3_d = nc.dram_tensor("W3", [H, 1], bf16, kind="ExternalInput")
    gbe_d = nc.dram_tensor("gbe", [128, 4], f32, kind="ExternalInput")  # g1,be1,g2,be2 cols
    iden_d = nc.dram_tensor("iden", [128, 128], bf16, kind="ExternalInput")
    iota_d = nc.dram_tensor("iota", [128, 128], bf16, kind="ExternalInput")
    onesr_d = nc.dram_tensor("onesr", [1, 128], bf16, kind="ExternalInput")
    out_d = nc.dram_tensor("out", [128, TILES], f32, kind="ExternalOutput")

    tbl_loc = [nc.dram_tensor(f"tbl{L}_loc", [NL, H], bf16) for L in range(3)]
    tbl_full = [nc.dram_tensor(f"tbl{L}_full", [TBLROWS, H], bf16, addr_space="Shared")
                for L in range(3)]
    st_in = [nc.dram_tensor(f"st{L}_in", [128, 2], f32) for L in range(2)]
    hpre_dump = None
    if debug_dumps:
        hpre_dump = [nc.dram_tensor(f"hpre{L}_dump", [128, NL], bf16, kind="ExternalOutput")
                     for L in range(2)]
    st_out = [nc.dram_tensor(f"st{L}_out", [128, 2], f32, addr_space="Shared")
              for L in range(2)]

    b3 = float(params["b3"][0])
    rg = [list(range(C))]

    from contextlib import ExitStack
    with tile.TileContext(nc) as tc, ExitStack() as ctx:
        res = ctx.enter_context(tc.tile_pool(name="res", bufs=1))
        mtp = ctx.enter_context(tc.tile_pool(name="mtp", bufs=3))
        stg = ctx.enter_context(tc.tile_pool(name="stg", bufs=4))
        msg = ctx.enter_context(tc.tile_pool(name="msg", bufs=num_msg_bufs))
        ohp = ctx.enter_context(tc.tile_pool(name="ohp", bufs=6))
        hpp = ctx.enter_context(tc.tile_pool(name="hpp", bufs=6))
        sqp = ctx.enter_context(tc.tile_pool(name="sqp", bufs=4))
        colp = ctx.enter_context(tc.tile_pool(name="colp", bufs=4))
        psA = ctx.enter_context(tc.tile_pool(name="psA", bufs=1, space="PSUM"))
        psB = ctx.enter_context(tc.tile_pool(name="psB", bufs=2, space="PSUM"))
        psACC = ctx.enter_context(tc.tile_pool(name="psACC", bufs=1, space="PSUM"))
        psST = ctx.enter_context(tc.tile_pool(name="psST", bufs=1, space="PSUM"))
        if True:
            # ---- residents ----
            gidx = res.tile([128, GIDX_COLS], i16, tag="gidx")
            nc.sync.dma_start(out=gidx[:], in_=gidx_d[:, :])
            dloc = res.tile([128, NOPS], bf16, tag="dloc")
            nc.sync.dma_start(out=dloc[:], in_=dloc_d[:, :])
            W1 = res.tile([IN, H], bf16, tag="W1")
            nc.sync.dma_start(out=W1[:], in_=W1_d[:, :])
            W2 = res.tile([H, H], bf16, tag="W2")
            nc.sync.dma_start(out=W2[:], in_=W2_d[:, :])
            W3 = res.tile([H, 1], bf16, tag="W3")
            nc.sync.dma_start(out=W3[:], in_=W3_d[:, :])
            gbe = res.tile([128, 4], f32, tag="gbe")
            nc.sync.dma_start(out=gbe[:], in_=gbe_d[:, :])
            iden = res.tile([128, 128], bf16, tag="iden")
            nc.sync.dma_start(out=iden[:], in_=iden_d[:, :])
            iota = res.tile([128, 128], bf16, tag="iota")
            nc.sync.dma_start(out=iota[:], in_=iota_d[:, :])
            onesr = res.tile([1, 128], bf16, tag="onesr")
            nc.sync.dma_start(out=onesr[:], in_=onesr_d[:, :])
            maskc = res.tile([128, TILES], bf16, tag="maskc")
            nc.sync.dma_start(out=maskc[:], in_=maskc_d[:, :])
            degc = res.tile([128, TILES], f32, tag="degc")
            nc.sync.dma_start(out=degc[:], in_=degc_d[:, :])

            # dis (node-major cols): dis = sqrt(1/deg)
            disc = res.tile([128, TILES], f32, tag="disc")
            nc.vector.reciprocal(out=disc[:], in_=degc[:])
            nc.scalar.sqrt(out=disc[:], in_=disc[:])

            NSL_ = NL // 512 if NL % 512 == 0 else NL // 512 + 1
            def mkchunks(prefix):
                out = []
                for j in range(NSL_):
                    w = min(512, NL - j * 512)
                    out.append(res.tile([128, w], bf16, tag=f"{prefix}{j}",
                                        name=f"{prefix}{j}"))
                return out
            hta = mkchunks("hta")
            htb = mkchunks("htb")
            hpre_t = [res.tile([128, 128], bf16, tag=f"hpre{t}", name=f"hpre{t}")
                      for t in range(TILES)]
            o_sb = res.tile([128, TILES], f32, tag="o_sb")
            # replicated iota (contiguous in0 for the one-hot op)
            MAXCNT = max(
                sum(1 for o in ops if o[0] == b) for b in range(NBAT))
            iorep = res.tile([128, MAXCNT, 128], bf16, tag="iorep")
            nc.vector.tensor_copy(
                out=iorep[:],
                in_=iota[:].rearrange("p (o f) -> p o f", o=1).broadcast_to([128, MAXCNT, 128]))

            NSL = NL // 512 if NL % 512 == 0 else NL // 512 + 1

            def phase_A(L, hin, Wt):
                """project + transpose + dis-scale -> table shard -> AllGather"""
                if L < 2:
                    for j in range(NSL):
                        a, b = j * 512, min((j + 1) * 512, NL)
                        if L == 0:
                            xc = mtp.tile([IN, 512], bf16, tag="xc")
                            nc.sync.dma_start(out=xc[:, : b - a], in_=xT_d[:, a:b])
                            rhs = xc[:, : b - a]
                        else:
                            rhs = hin[j][:, : b - a]
                        pa = psA.tile([128, 512], f32, tag="psA")
                        nc.tensor.matmul(pa[:, : b - a], Wt[:], rhs,
                                         start=True, stop=True)
                        mt = mtp.tile([128, 512], bf16, tag="mt")
                        nc.scalar.activation(mt[:, : b - a], pa[:, : b - a],
                                             mybir.ActivationFunctionType.Copy)
                        for jj in range((b - a) // 128):
                            t = (a // 128) + jj
                            pb = psB.tile([128, 128], f32, tag="psB")
                            nc.tensor.matmul(pb[:], mt[:, jj * 128:(jj + 1) * 128],
                                             iden[:], start=True, stop=True)
                            sg = stg.tile([128, 128], bf16, tag="stg")
                            nc.vector.tensor_scalar_mul(
                                out=sg[:], in0=pb[:], scalar1=disc[:, t: t + 1])
                            nc.sync.dma_start(out=tbl_loc[L][t * 128:(t + 1) * 128, :],
                                              in_=sg[:])
                else:
                    # L3: m3 = W3^T @ h2T; scale by dis after transpose; replicate rows
                    for j in range(NSL):
                        a, b = j * 512, min((j + 1) * 512, NL)
                        pa = psA.tile([1, 512], f32, tag="psA")
                        nc.tensor.matmul(pa[:, : b - a], Wt[:], hin[j][:, : b - a],
                                         start=True, stop=True)
                        m3c = mtp.tile([1, 512], bf16, tag="m3c")
                        nc.scalar.activation(m3c[:, : b - a], pa[:, : b - a],
                                             mybir.ActivationFunctionType.Copy)
                        for jj in range((b - a) // 128):
                            t = (a // 128) + jj
                            pb = psB.tile([128, 128], f32, tag="psB")
                            nc.tensor.matmul(pb[:], m3c[:, jj * 128:(jj + 1) * 128],
                                             onesr[:], start=True, stop=True)
                            sg = stg.tile([128, 128], bf16, tag="stg")
                            nc.vector.tensor_scalar_mul(
                                out=sg[:], in0=pb[:], scalar1=disc[:, t: t + 1])
                            nc.sync.dma_start(out=tbl_loc[L][t * 128:(t + 1) * 128, :],
                                              in_=sg[:])
                nc.gpsimd.collective_compute(
                    "AllGather", mybir.AluOpType.bypass,
                    ins=[tbl_loc[L][:]], outs=[tbl_full[L][:]], replica_groups=rg)

            def phase_CD(L):
                ps_of_grp = {}
                sts = None
                if L < 2:
                    sts = psST.tile([128, 2], f32, tag="sts")
                    nc.vector.memset(sts[:], 0.0)
                # ops are contiguous per batch
                ops_span = {}
                for o_id, (bb, _, _) in enumerate(ops):
                    if bb not in ops_span:
                        ops_span[bb] = [o_id, 0]
                    ops_span[bb][1] += 1
                for bb in range(NBAT):
                    g, k, size, off = batches[bb]
                    if g not in ps_of_grp:
                        ps_of_grp[g] = psACC.tile([128, G, 128], f32, tag="acc", name=f"accL{L}g{g}")
                        nc.vector.memset(ps_of_grp[g][:], 0.0)
                    m = msg.tile([128, NI_MAX // 128, H], bf16, tag="msg")
                    co = col_off[bb]
                    base = (CHUNK * k)
                    rows = min(CHUNK, TBLROWS - base)
                    nc.gpsimd.dma_gather(
                        out_ap=m[:, : size // 128, :],
                        in_ap=tbl_full[L][base: base + rows, :],
                        idxs_ap=gidx[:, co: co + size // 16],
                        num_idxs=size, num_idxs_reg=size, elem_size=H,
                        queue_num=bb % 4)
                    o0, cnt = ops_span[bb]
                    oh = ohp.tile([128, cnt, 128], bf16, tag="oh", name=f"ohL{L}b{bb}")
                    dloc_b = dloc[:, o0: o0 + cnt].rearrange("p (o f) -> p o f", f=1).broadcast_to([128, cnt, 128])
                    nc.vector.tensor_tensor(out=oh[:], in0=iorep[:, : cnt, :], in1=dloc_b,
                                            op=mybir.AluOpType.is_equal)
                    pst = ps_of_grp[g]
                    for j in range(cnt):
                        o_id = o0 + j
                        _, col, t = ops[o_id]
                        ti = t - g * G
                        nc.tensor.matmul(pst[:, ti, :], oh[:, j, :], m[:, col, :],
                                         start=False, stop=False,
                                         skip_group_check=True)
                        if last_op[t] == o_id:
                            finish_tile(L, t, pst, ti, sts)
                # after all tiles
                if L < 2:
                    finish_layer_bn(L, sts)
                else:
                    nc.sync.dma_start(out=out_d[:, :], in_=o_sb[:])

            def finish_tile(L, t, pst, ti, sts):
                # self-loop term: acc_t += diag(dis^2)_t @ tbl_tile_t
                tbt = stg.tile([128, 128], bf16, tag="tbt", name=f"tbtL{L}t{t}")
                nc.sync.dma_start(out=tbt[:], in_=tbl_loc[L][t * 128:(t + 1) * 128, :])
                nc.tensor.matmul(pst[:, ti, :], iden[:], tbt[:],
                                 start=False, stop=False, skip_group_check=True)
                if L == 2:
                    nc.scalar.activation(o_sb[:, t: t + 1], pst[:, ti, 0:1],
                                         mybir.ActivationFunctionType.Sigmoid,
                                         scale=disc[:, t: t + 1], bias=b3)
                    return
                hp = hpp.tile([128, 128], bf16, tag="hp")
                nc.scalar.activation(hp[:], pst[:, ti, :],
                                     mybir.ActivationFunctionType.Copy,
                                     scale=disc[:, t: t + 1])
                sq = sqp.tile([128, 128], bf16, tag="sq")
                nc.scalar.activation(sq[:], hp[:],
                                     mybir.ActivationFunctionType.Square)
                nc.tensor.matmul(sts[:, 0:1], hp[:], maskc[:, t: t + 1],
                                 start=False, stop=False, skip_group_check=True)
                nc.tensor.matmul(sts[:, 1:2], sq[:], maskc[:, t: t + 1],
                                 start=False, stop=False, skip_group_check=True)
                pb = psB.tile([128, 128], f32, tag="psB")
                nc.tensor.matmul(pb[:], hp[:], iden[:], start=True, stop=True)
                nc.scalar.activation(hpre_t[t][:], pb[:],
                                     mybir.ActivationFunctionType.Copy)

            def finish_layer_bn(L, sts):
                stat = colp.tile([128, 2], f32, tag="stat")
                nc.vector.tensor_copy(out=stat[:], in_=sts[:])
                nc.sync.dma_start(out=st_in[L][:, :], in_=stat[:])
                nc.gpsimd.collective_compute(
                    "AllReduce", mybir.AluOpType.add,
                    ins=[st_in[L][:]], outs=[st_out[L][:]], replica_groups=rg)
                stg_ = colp.tile([128, 2], f32, tag="statg")
                nc.sync.dma_start(out=stg_[:], in_=st_out[L][:, :])
                mu = colp.tile([128, 4], f32, tag="mu")
                inv_n = 1.0 / float(N)
                # mu = s1/N ; ms2 = s2/N
                nc.vector.tensor_scalar_mul(out=mu[:, 0:2], in0=stg_[:], scalar1=inv_n)
                # var = ms2 - mu^2 -> mu[:,2]
                nc.vector.tensor_tensor(out=mu[:, 2:3], in0=mu[:, 0:1], in1=mu[:, 0:1],
                                        op=mybir.AluOpType.mult)
                nc.vector.tensor_tensor(out=mu[:, 2:3], in0=mu[:, 1:2], in1=mu[:, 2:3],
                                        op=mybir.AluOpType.subtract)
                nc.vector.tensor_scalar_add(out=mu[:, 2:3], in0=mu[:, 2:3],
                                            scalar1=BN_EPS)
                # rstd -> mu[:,3] = sqrt(1/(var+eps))
                nc.vector.reciprocal(out=mu[:, 3:4], in_=mu[:, 2:3])
                nc.scalar.sqrt(out=mu[:, 3:4], in_=mu[:, 3:4])
                AB = colp.tile([128, 2], f32, tag="AB")
                gcol = gbe[:, 2 * L: 2 * L + 1]
                becol = gbe[:, 2 * L + 1: 2 * L + 2]
                nc.vector.tensor_tensor(out=AB[:, 0:1], in0=gcol, in1=mu[:, 3:4],
                                        op=mybir.AluOpType.mult)
                nc.vector.tensor_tensor(out=AB[:, 1:2], in0=mu[:, 0:1], in1=AB[:, 0:1],
                                        op=mybir.AluOpType.mult)
                nc.vector.tensor_tensor(out=AB[:, 1:2], in0=becol, in1=AB[:, 1:2],
                                        op=mybir.AluOpType.subtract)
                hout = htb if L == 0 else hta
                for t in range(TILES):
                    j, r = t // 4, t % 4
                    nc.scalar.activation(hout[j][:, r * 128:(r + 1) * 128],
                                         hpre_t[t][:],
                                         mybir.ActivationFunctionType.Relu,
                                         scale=AB[:, 0:1], bias=AB[:, 1:2])

            # ---- run 3 layers ----
            for L in range(3):
                hin = None if L == 0 else (htb if L == 1 else hta)
                Wt = [W1, W2, W3][L]
                emitted_b = [0]
                msg_of_bat = {}
                grp_left = {}
                for (_, _, t) in ops:
                    gg_ = t // G
                    pass
                from collections import Counter as _C
                tiles_per_grp = _C()
                for t_ in range(TILES):
                    tiles_per_grp[t_ // G] += 1
                grp_left = dict(tiles_per_grp)
                hp_of_tile = {}
                phase_A(L, hin, Wt)
                phase_CD(L)

    nc.finalize()
    return nc


def make_inputs(cfg, prep, inputs, core):
    """Per-core input map."""
    N, NPC, NL, IN = cfg["N"], cfg["NPC"], cfg["NL"], cfg["IN"]
    TILES = NL // 128
    bf = ml_dtypes.bfloat16
    x = np.asarray(inputs["x"], np.float32)
    deg = prep["deg"]

    xl = np.zeros((NL, IN), np.float32)
    xl[:NPC] = x[core * NPC:(core + 1) * NPC]
    degl = np.ones(NL, np.float32)
    degl[:NPC] = deg[core * NPC:(core + 1) * NPC]
    mask = np.zeros(NL, np.float32)
    mask[:NPC] = 1.0

    gbe = np.stack([
        np.asarray(inputs["g1"], np.float32), np.asarray(inputs["be1"], np.float32),
        np.asarray(inputs["g2"], np.float32), np.asarray(inputs["be2"], np.float32),
    ], axis=1)  # (128, 4)

    return {
        "xT": xl.T.astype(bf).copy(),
        "degc": degl.reshape(TILES, 128).T.copy(),
        "maskc": mask.reshape(TILES, 128).T.astype(bf).copy(),
        "gidx": prep["gidx_data"][core],
        "dloc": prep["dloc_data"][core].astype(bf),
        "W1": np.asarray(inputs["W1"], np.float32).astype(bf),
        "W2": np.asarray(inputs["W2"], np.float32).astype(bf),
        "W3": np.asarray(inputs["W3"], np.float32).astype(bf),
        "gbe": gbe,
        "iden": np.eye(128, dtype=np.float32).astype(bf),
        "iota": np.tile(np.arange(128, dtype=np.float32), (128, 1)).astype(bf),
        "onesr": np.ones((1, 128), np.float32).astype(bf),
    }


def unshard_output(cfg, results):
    N, NPC, NL = cfg["N"], cfg["NPC"], cfg["NL"]
    TILES = NL // 128
    out = np.zeros((N, 1), np.float32)
    for c in range(C):
        o = results[c]["out"]            # (128, TILES)
        flat = o.T.reshape(NL)           # node-major
        out[c * NPC:(c + 1) * NPC, 0] = flat[:NPC]
    return out


def _ensure_axon_hooks_shim():
    """bass_utils' trace path imports antenv.axon_hooks, which this image
    lacks; register a no-op so a stray BASS_TRACE=1 can't crash the run."""
    import types
    if 'antenv.axon_hooks' in sys.modules:
        return
    try:
        import antenv
        from antenv import axon_hooks  # noqa: F401
    except ImportError:
        mod = types.ModuleType('antenv.axon_hooks')
        _hook = [None]
        mod.set_axon_ntff_profile_hook = lambda h: _hook.__setitem__(0, h)
        mod.get_axon_ntff_profile_hook = lambda: _hook[0]
        sys.modules['antenv.axon_hooks'] = mod
        try:
            antenv.axon_hooks = mod
        except Exception:
            pass


def kernel(**inputs):
    import os
    import numpy as np
    from concourse import bass_utils

    _ensure_axon_hooks_shim()
    cfg = make_cfg(full=True)
    inputs = {k: np.asarray(v) for k, v in inputs.items()}
    prep = host_prep(cfg, inputs["edge_index"])
    nc = build_graph(cfg, prep, inputs)
    in_maps = [make_inputs(cfg, prep, inputs, c) for c in range(C)]
    prev = os.environ.get("BASS_NEVER_TRACE")
    os.environ["BASS_NEVER_TRACE"] = "1"
    try:
        res = bass_utils.run_bass_kernel_spmd(nc, in_maps, list(range(C)), trace=False)
    finally:
        if prev is None:
            os.environ.pop("BASS_NEVER_TRACE", None)
        else:
            os.environ["BASS_NEVER_TRACE"] = prev
    return unshard_output(cfg, [res.results[c] for c in range(C)])

